# revision 16
# baseline (speedup 1.0000x reference)
"""Trainium2 Bass kernel: dual-softmax cross-attention bilinear forms.

Math (per batch b, a = corr[b] in [N, N], N = 3072):
    attn = exp(2a) * (1/rowsum_a) outer (1/colsum_a)
    fund1 = v1^T attn v1,  fund2^T = v2^T attn^T v2
Device computes, per core (4 batches x 2 row-halves = 8 cores), for its
half slab [NH=1536, N]:
    E1 = exp(a + B) fp16 on the scalar engine (a arrives as fp8e4m3 --
    the quantization noise washes out through the dual normalization).
    rowsum via the activation accumulator; colsum partials via
    ones^T @ E1 fp16 matmuls spread over the PE's 4 column groups.
    e2 = fp8(E1^2 * rinv_row). For most tiles the DVE squares into fp16
    at 2x_1P (~1.7us) and a GPSIMD-initiated casting DMA converts fp16
    -> fp8 (bit-exact RNE, verified) -- this halves the DVE's square
    cost vs the direct fp8 write (1x) and frees it for mid-stream psum
    exports. The last two tiles square directly to fp8 on the DVE so
    the cast latency is off the critical path.
    X = e2^T @ vr -- fp8 DoubleRow matmul trios per (m, ipair):
    [0:256 | 256:512 | pos 16-wide window in a shared bank], ~278ns
    warm. PSUM is the scarce resource (8 banks): 2 colsum + 1 pos +
    3 resident m-tiles + 2 rotating. Resident m's eat each ipair as it
    lands; PARKED m's run a pairs-0..3 (or 0..4) burst mid-stream into
    a rotating bank, export to SBUF on the freed DVE, and finish with a
    2-(or 1-)ipair burst + add-export after the stream. Remaining m's
    run full 6-ipair bursts post-stream on the hot PE.
Host finishes: colsum normalization + the small [N,262] bilinear GEMMs.
"""

import numpy as np

import concourse.tile as tile
from concourse import bacc, bass_utils, mybir

B, N, C = 4, 3072, 256
H, W = 48, 64
CP = C + 6          # 262
CX = 2 * C          # 512: [x1 256 | x2 256]
NH = N // 2         # 1536 rows per core
NT = NH // 128      # 12 row tiles per core
NP = NT // 2        # 6 DoubleRow ipairs
MT = N // 128       # 24 column tiles
CS_CHUNK = 512
NCS = N // CS_CHUNK  # 6 colsum psum chunks
CVP = CX + 16        # 528: fp8 v row: [x1 256 | x2 256 | pos 6 | pad 10]
B_SHIFT = 2.875      # E1 = exp(a + B_SHIFT); constants cancel on host

RES = 3              # m-tiles psum-resident through the stream (m 0..2)
P1 = 5               # parked group 1 (pairs 0-3 mid-stream): m 3..7
P2 = 5               # parked group 2 (pairs 0-4 mid-stream): m 8..12
DIRECT_FP8 = (NT - 2, NT - 1)   # tiles squared straight to fp8 (no cast)

FP32 = mybir.dt.float32
FP16 = mybir.dt.float16
FP8 = mybir.dt.float8e4
DR = mybir.MatmulPerfMode.DoubleRow
MUL = mybir.AluOpType.mult

TRACE = False
LAST_RESULT = None
_CACHED_NC = None


def _build_kernel():
    nc = bacc.Bacc("TRN2", target_bir_lowering=False, debug=False)
    a_in = nc.dram_tensor("a_half", [NH, N], FP8, kind="ExternalInput").ap()
    v_in = nc.dram_tensor("v_half", [128, NT * CVP], FP8, kind="ExternalInput").ap()
    x_out = nc.dram_tensor("x_out", [128, MT * CX], FP16, kind="ExternalOutput").ap()
    pos_out = nc.dram_tensor("pos_out", [128, MT * 16], FP32, kind="ExternalOutput").ap()
    cs_out = nc.dram_tensor("cs_out", [8, CS_CHUNK], FP32, kind="ExternalOutput").ap()

    with tile.TileContext(nc) as tc:
        _kernel_body(tc, a_in, v_in, x_out, pos_out, cs_out)
    nc.compile()
    return nc


def _kernel_body(tc, a_in, v_in, x_out, pos_out, cs_out):
    nc = tc.nc
    with (
        tc.tile_pool(name="singles", bufs=1) as singles,
        tc.tile_pool(name="a_pool", bufs=4) as a_pool,
        tc.tile_pool(name="e_pool", bufs=5) as e_pool,
        tc.tile_pool(name="e16_pool", bufs=3) as e16_pool,
        tc.tile_pool(name="cs_psum", bufs=1, space="PSUM") as cs_psum,
        tc.tile_pool(name="pos_psum", bufs=1, space="PSUM") as pos_psum,
        tc.tile_pool(name="x_psum", bufs=RES + 2, space="PSUM") as x_psum,
    ):
        ones_t = singles.tile([128, 1], FP16)
        nc.vector.memset(ones_t, 1.0)
        bias_t = singles.tile([128, 1], FP32)
        nc.vector.memset(bias_t, B_SHIFT)

        # prefetch the exp table-set off the critical path
        dummy_t = singles.tile([128, 1], FP32)
        nc.scalar.activation(
            out=dummy_t, in_=bias_t, func=mybir.ActivationFunctionType.Exp
        )

        vr_all = singles.tile([128, NT, CVP], FP8)
        e2_all = singles.tile([128, NT, N], FP8)
        rowsum_all = singles.tile([128, NT + 4], FP32)
        rinv_all = singles.tile([128, NT], FP32)
        x_sb = singles.tile([128, MT, CX], FP16)
        pos_sb = singles.tile([128, MT * 16], FP32)
        cs_sb = singles.tile([128, 2, CS_CHUNK], FP32)

        cs_bank = [
            cs_psum.tile([128, CS_CHUNK], FP32, name=f"csb{t}", tag=f"csb{t}")
            for t in range(2)
        ]
        pos_bank = pos_psum.tile([128, CS_CHUNK], FP32, name="posb", tag="posb")
        for t in range(2):
            nc.vector.memset(cs_bank[t], 0.0)
        nc.vector.memset(pos_bank, 0.0)

        def cs_ap(j):
            t, p = divmod(j, 4)
            return cs_bank[t][32 * p : 32 * p + 1, :]

        def stream_chunk(i, e_t, col_lo, col_hi, accum_col):
            a_t = a_pool.tile([128, N], FP8, name="a_t", tag="a_t")
            nc.sync.dma_start(
                out=a_t[:, col_lo:col_hi],
                in_=a_in[i * 128 : (i + 1) * 128, col_lo:col_hi],
            )
            nc.scalar.activation(
                out=e_t[:, col_lo:col_hi],
                in_=a_t[:, col_lo:col_hi],
                func=mybir.ActivationFunctionType.Exp,
                bias=bias_t,
                scale=1.0,
                accum_out=rowsum_all[:, accum_col : accum_col + 1],
            )
            for j in range(col_lo // CS_CHUNK, col_hi // CS_CHUNK):
                nc.tensor.matmul(
                    cs_ap(j),
                    lhsT=ones_t,
                    rhs=e_t[:, j * CS_CHUNK : (j + 1) * CS_CHUNK],
                    start=False,
                    stop=(i == NT - 1),
                    skip_group_check=True,
                    tile_position=(0, 32 * (j % 4)),
                )

        def square_tile(i, e_t):
            nc.vector.reciprocal(rinv_all[:, i : i + 1], rowsum_all[:, i : i + 1])
            if i in DIRECT_FP8:
                # straight to fp8 (1x DVE) -- no cast-DMA latency at the end
                nc.vector.scalar_tensor_tensor(
                    out=e2_all[:, i, :],
                    in0=e_t, scalar=rinv_all[:, i : i + 1], in1=e_t,
                    op0=MUL, op1=MUL,
                )
            else:
                # fp16 out -> DVE runs 2x_1P; GPSIMD casting DMA -> fp8
                e16 = e16_pool.tile([128, N], FP16, name="e16", tag="e16")
                nc.vector.scalar_tensor_tensor(
                    out=e16,
                    in0=e_t, scalar=rinv_all[:, i : i + 1], in1=e_t,
                    op0=MUL, op1=MUL,
                )
                nc.gpsimd.dma_start(out=e2_all[:, i, :], in_=e16)

        def gemm_trio(m, p, xp, first, last, pos_last):
            lhsT = e2_all[:, 2 * p : 2 * p + 2, m * 128 : (m + 1) * 128]
            nc.tensor.matmul(
                xp[:, 0:256],
                lhsT=lhsT,
                rhs=vr_all[:, 2 * p : 2 * p + 2, 0:256],
                start=first, stop=last, perf_mode=DR, skip_group_check=True,
            )
            nc.tensor.matmul(
                xp[:, 256:512],
                lhsT=lhsT,
                rhs=vr_all[:, 2 * p : 2 * p + 2, 256:512],
                start=False, stop=last, perf_mode=DR, skip_group_check=True,
            )
            nc.tensor.matmul(
                pos_bank[:, 16 * m : 16 * (m + 1)],
                lhsT=lhsT,
                rhs=vr_all[:, 2 * p : 2 * p + 2, CX : CX + 16],
                start=False, stop=pos_last, perf_mode=DR, skip_group_check=True,
            )

        # ---- streaming phase ----------------------------------------
        res_xp = [
            x_psum.tile([128, CX], FP32, name="xp", tag="xp")
            for m in range(RES)
        ]
        parked1 = list(range(RES, RES + P1))
        parked2 = list(range(RES + P1, RES + P1 + P2))

        for i in range(NT):
            e_t = e_pool.tile([128, N], FP16, name="e_t", tag="e_t")
            if i == 0:
                # tile 0 in halves: the first exp starts sooner
                stream_chunk(0, e_t, 0, N // 2, 0)
                stream_chunk(0, e_t, N // 2, N, NT)
                nc.vector.tensor_add(
                    rowsum_all[:, 0:1], rowsum_all[:, 0:1],
                    rowsum_all[:, NT : NT + 1],
                )
                # v load rides after tile 0 so it cannot delay exp #2
                nc.sync.dma_start(out=vr_all, in_=v_in)
            else:
                stream_chunk(i, e_t, 0, N, i)
            square_tile(i, e_t)
            if i % 2 == 1:
                p = i // 2
                for m in range(RES):
                    gemm_trio(m, p, res_xp[m], first=(p == 0),
                              last=(p == NP - 1), pos_last=(p == NP - 1))
            if i == 7:
                # parked group 1: pairs 0..3 burst, export on the DVE
                for m in parked1:
                    xp = x_psum.tile([128, CX], FP32, name="xp", tag="xp")
                    for p in range(4):
                        gemm_trio(m, p, xp, first=(p == 0), last=(p == 3),
                                  pos_last=False)
                    nc.vector.tensor_copy(out=x_sb[:, m, :], in_=xp)
            if i == 9:
                # parked group 2: pairs 0..4 burst, export on the DVE
                for m in parked2:
                    xp = x_psum.tile([128, CX], FP32, name="xp", tag="xp")
                    for p in range(5):
                        gemm_trio(m, p, xp, first=(p == 0), last=(p == 4),
                                  pos_last=False)
                    nc.vector.tensor_copy(out=x_sb[:, m, :], in_=xp)

        # ---- tail ------------------------------------------------------
        for t in range(2):
            nc.scalar.copy(out=cs_sb[:, t, :], in_=cs_bank[t])
            nc.sync.dma_start(
                out=cs_out[4 * t : 4 * t + 4, :], in_=cs_sb[0:128:32, t, :]
            )
        for m in range(RES):
            nc.scalar.copy(out=x_sb[:, m, :], in_=res_xp[m])

        stored = 0

        def store_upto(hi):
            nonlocal stored
            while stored + 2 <= hi:
                g = stored // 2
                nc.sync.dma_start(
                    out=x_out[:, 2 * g * CX : 2 * (g + 1) * CX],
                    in_=x_sb[:, 2 * g : 2 * (g + 1), :],
                )
                stored += 2

        # parked finishers: group 1 needs pairs 4,5; group 2 pair 5 only
        for m in parked1:
            xp = x_psum.tile([128, CX], FP32, name="xp", tag="xp")
            for p in (4, 5):
                gemm_trio(m, p, xp, first=(p == 4), last=(p == 5),
                          pos_last=(p == NP - 1))
            nc.vector.tensor_add(x_sb[:, m, :], xp, x_sb[:, m, :])
        for m in parked2:
            xp = x_psum.tile([128, CX], FP32, name="xp", tag="xp")
            gemm_trio(m, 5, xp, first=True, last=True, pos_last=True)
            nc.vector.tensor_add(x_sb[:, m, :], xp, x_sb[:, m, :])
        store_upto(RES + P1 + P2)

        for k, m in enumerate(range(RES + P1 + P2, MT)):
            xp = x_psum.tile([128, CX], FP32, name="xp", tag="xp")
            for p in range(NP):
                gemm_trio(m, p, xp, first=(p == 0), last=(p == NP - 1),
                          pos_last=(p == NP - 1))
            if m % 2 == 0:
                nc.scalar.copy(out=x_sb[:, m, :], in_=xp)
            else:
                nc.vector.tensor_copy(out=x_sb[:, m, :], in_=xp)
            store_upto(m)
        store_upto(MT)

        # pos bank: single export + store
        nc.scalar.copy(out=pos_sb, in_=pos_bank[:, 0 : MT * 16])
        nc.sync.dma_start(out=pos_out, in_=pos_sb)


def _positional_encodings():
    ys = np.linspace(-1.0, 1.0, H, dtype=np.float32)
    xs = np.linspace(-1.0, 1.0, W, dtype=np.float32)
    p3 = np.tile(ys, W)
    p4 = np.repeat(xs, H)
    pos = np.stack([p3 * p3, p4 * p4, p3 * p4, p3, p4, np.ones_like(p3)], axis=-1)
    return pos.astype(np.float32)  # [N, 6]


def kernel(x1, x2, corr, W_proj, b_proj):
    global _CACHED_NC, LAST_RESULT
    x1 = np.asarray(x1, dtype=np.float32)
    x2 = np.asarray(x2, dtype=np.float32)
    corr = np.asarray(corr, dtype=np.float32)
    W_proj = np.asarray(W_proj, dtype=np.float32)
    b_proj = np.asarray(b_proj, dtype=np.float32)

    import ml_dtypes

    pos = _positional_encodings()
    a = corr.reshape(B, N, N).astype(ml_dtypes.float8_e4m3)
    v_all = np.zeros((B, N, CVP), dtype=np.float32)
    v_all[:, :, 0:C] = x1
    v_all[:, :, C : 2 * C] = x2
    v_all[:, :, CX : CX + 6] = np.broadcast_to(pos, (B, N, 6))
    v_all = v_all.astype(ml_dtypes.float8_e4m3)

    if _CACHED_NC is None:
        _CACHED_NC = _build_kernel()
    nc = _CACHED_NC

    in_maps = []
    for b in range(B):
        for h in range(2):
            rows = slice(h * NH, (h + 1) * NH)
            vp = (
                v_all[b, rows, :]
                .reshape(NT, 128, CVP)
                .transpose(1, 0, 2)
                .reshape(128, NT * CVP)
            )
            in_maps.append(
                {
                    "a_half": np.ascontiguousarray(a[b, rows, :]),
                    "v_half": np.ascontiguousarray(vp),
                }
            )

    res = bass_utils.run_bass_kernel_spmd(
        nc, in_maps, core_ids=list(range(8)), trace=TRACE
    )
    LAST_RESULT = res

    v1 = np.concatenate([x1, np.broadcast_to(pos, (B, N, 6))], axis=2)
    v2 = np.concatenate([x2, np.broadcast_to(pos, (B, N, 6))], axis=2)

    out1 = np.empty((B, CP, C), dtype=np.float32)
    out2 = np.empty((B, CP, C), dtype=np.float32)
    for b in range(B):
        r0, r1 = res.results[2 * b], res.results[2 * b + 1]
        X = (
            r0["x_out"].astype(np.float32) + r1["x_out"].astype(np.float32)
        ).reshape(128, MT, CX).transpose(1, 0, 2).reshape(N, CX)
        pos_raw = r0["pos_out"] + r1["pos_out"]   # [128, MT*16]
        pos_x = (
            pos_raw.reshape(128, MT, 16)[:, :, 0:6]
            .transpose(1, 0, 2)
            .reshape(N, 6)
        )
        colsum = np.empty(N, dtype=np.float32)
        for j in range(NCS):
            t, p = divmod(j, 4)
            colsum[j * CS_CHUNK : (j + 1) * CS_CHUNK] = (
                r0["cs_out"][4 * t + p] + r1["cs_out"][4 * t + p]
            )
        c = 1.0 / colsum
        vc1 = v1[b] * c[:, None]
        vc2 = v2[b] * c[:, None]
        X1 = np.concatenate([X[:, 0:256], pos_x], axis=1)   # [N, 262]
        X2 = np.concatenate([X[:, 256:512], pos_x], axis=1)
        fund1 = X1.T @ vc1      # [262, 262] = v1^T attn v1
        fund2t = X2.T @ vc2     # = (v2^T attn^T v2)^T
        out1[b] = fund1.T @ W_proj + b_proj
        out2[b] = fund2t @ W_proj + b_proj
    return (out2, out1)
